# revision 1
# baseline (speedup 1.0000x reference)
"""Trainium2 Bass kernel for a 3-layer GRU (B=512, T=1000, H=64, OUT=300).

Strategy:
- Data-parallel over batch: 8 cores x 64 rows each; weights replicated.
- Per core, everything is kept in a transposed "gate-major" layout:
  state h is [H, B] so matmuls are out[gates, B] = W_aug.T @ [h; 1].
- The 3 layers are software-pipelined with a one-tick skew: at tick k,
  layer 0 consumes x[k], layer 1 consumes h0 produced at tick k-1, layer 2
  consumes h1 produced at tick k-1.  All three layers' gate tensors are
  stacked along the free dim so each elementwise/activation instruction
  covers all layers at once.
- Biases ride inside the matmuls via an "ones row" appended to the state
  tile (K=65 aug matmuls).  z-gate weights are negated so the sigmoid
  directly produces w = 1-z:  h' = h + w*(n - h).
- Layer-0's scalar input enters via one-hot weight matmuls against a
  time-major x tile (K=32, tile_position row groups).
"""

import os
import sys
import numpy as np

sys.path.insert(0, "/opt/trn_rl_repo")

B_FULL, T, H, OUT, L = 512, 1000, 64, 300, 3
NCORES = 8
B = B_FULL // NCORES           # 64 batch rows per core
NT = T + 2                     # pipeline ticks (2 warmup skew ticks)
TPAD = 1024                    # x padded to 8 blocks of 128 ticks

_cache = {}


def _build_weights_np(inputs, np_dt):
    """Pack all weights into the two host-side arrays the kernel DMAs in.

    Returns (wts [65, WC], wx [128, 32, 192], offsets dict).
    Gate-column convention for 'rz' blocks: cols 0:64 are r rows, cols
    64:128 are NEGATED z rows (so sigmoid yields 1-z).
    """
    offs = {}
    blocks = []
    col = 0

    def add(name, arr):
        nonlocal col
        assert arr.shape[0] == 65
        offs[name] = (col, arr.shape[1])
        blocks.append(arr)
        col += arr.shape[1]

    def rz_lhsT(W, b_total):
        # W: [192, in_dim] torch layout (r rows 0:64, z rows 64:128)
        # returns [in_dim+1, 128] lhsT with aug bias row (z part negated)
        Wrz = np.concatenate([W[0:64], -W[64:128]], axis=0)      # [128, in]
        aug = np.concatenate([b_total[0:64], -b_total[64:128]])  # [128]
        return np.concatenate([Wrz.T, aug[None, :]], axis=0)     # [in+1,128]

    def n_lhsT(W, b):
        Wn = W[128:192]                                          # [64, in]
        return np.concatenate([Wn.T, b[128:192][None, :]], axis=0)

    def pad65(a):
        if a.shape[0] < 65:
            a = np.concatenate(
                [a, np.zeros((65 - a.shape[0], a.shape[1]), a.dtype)], axis=0)
        return a

    # --- layer 0 (input is scalar x, handled by one-hot matmuls) ---
    Wh0, bi0, bh0 = inputs["W_hh0"], inputs["b_ih0"], inputs["b_hh0"]
    add("Wh0_rz", rz_lhsT(Wh0, bi0 + bh0))          # [65, 128] full rz bias
    add("Wh0_n", n_lhsT(Wh0, bh0))                  # [65, 64]
    b0n = np.zeros((65, 64), np.float64)
    b0n[0, :] = bi0[128:192]                        # K=1 bias matmul row
    add("b0_n", b0n)

    for l in (1, 2):
        Wi, Wh = inputs[f"W_ih{l}"], inputs[f"W_hh{l}"]
        bi, bh = inputs[f"b_ih{l}"], inputs[f"b_hh{l}"]
        add(f"Wi_rz{l}", rz_lhsT(Wi, bi + bh))      # aug carries total bias
        add(f"Wh_rz{l}", pad65(rz_lhsT(Wh, np.zeros(192))[0:64]))  # no aug
        add(f"Wi_n{l}", n_lhsT(Wi, bi))
        add(f"Wh_n{l}", n_lhsT(Wh, bh))

    fc_w, fc_b = inputs["fc_w"], inputs["fc_b"]     # [300, 64], [300]
    add("fc", np.concatenate([fc_w.T, fc_b[None, :]], axis=0))  # [65, 300]

    wts = np.concatenate(blocks, axis=1).astype(np_dt)

    # one-hot x weights: wx[row, q, m] = (row == q) * val[m], replicated on
    # all four 32-partition groups.
    Wi0 = inputs["W_ih0"][:, 0]                     # [192]
    val = np.concatenate([Wi0[0:64], -Wi0[64:128], Wi0[128:192]])
    wx = np.zeros((32, 32, 192), np.float64)
    wx[np.arange(32), np.arange(32), :] = val[None, :]
    wx = wx.astype(np_dt)                           # [32, 32, 192]
    return wts, wx, offs


def _build_program(mm_np_dt=np.float32, n_ticks=NT):
    import concourse.bass as bass
    import concourse.tile as tile
    import concourse.bacc as bacc
    from concourse import mybir
    from contextlib import ExitStack

    F32 = mybir.dt.float32
    MM = mybir.dt.from_np(np.dtype(mm_np_dt))
    AF = mybir.ActivationFunctionType
    OP = mybir.AluOpType

    # offsets must match _build_weights_np; rebuild cheaply with zeros
    dummy = {k: np.zeros(v) for k, v in {
        "W_hh0": (192, 64), "b_ih0": (192,), "b_hh0": (192,),
        "W_ih1": (192, 64), "W_hh1": (192, 64), "b_ih1": (192,), "b_hh1": (192,),
        "W_ih2": (192, 64), "W_hh2": (192, 64), "b_ih2": (192,), "b_hh2": (192,),
        "W_ih0": (192, 1), "fc_w": (300, 64), "fc_b": (300,),
    }.items()}
    _, _, offs = _build_weights_np(dummy, np.float32)
    WC = sum(w for (_, w) in offs.values())

    nc = bacc.Bacc("TRN2", target_bir_lowering=False, debug=False,
                   num_devices=NCORES)

    t_xt = nc.dram_tensor("xt", [32, TPAD // 32, B], MM,
                          kind="ExternalInput").ap()
    t_wx = nc.dram_tensor("wx", [32, 32, 192], MM, kind="ExternalInput").ap()
    t_wts = nc.dram_tensor("wts", [65, WC], MM, kind="ExternalInput").ap()
    t_out = nc.dram_tensor("out", [OUT, B], F32, kind="ExternalOutput").ap()

    def w_ap(sb, name, rows=65):
        o, w = offs[name]
        return sb[0:rows, o:o + w]

    with tile.TileContext(nc) as tc, ExitStack() as ctx:
        const = ctx.enter_context(tc.tile_pool(name="const", bufs=1))
        xt_sb = const.tile([32, TPAD // 32, B], MM, tag="xt")
        nc.sync.dma_start(out=xt_sb[:], in_=t_xt[:])
        wx_sb = const.tile([32, 32, 192], MM, tag="wx")
        nc.sync.dma_start(out=wx_sb[:], in_=t_wx[:])
        wts_sb = const.tile([65, WC], MM, tag="wts")
        nc.sync.dma_start(out=wts_sb[:], in_=t_wts[:])
        ones_sb = const.tile([1, B], MM, tag="ones")
        nc.vector.memset(ones_sb[:], 1.0)

        # Two independent batch streams of 32 rows each: their dependency
        # chains interleave on the engines, roughly doubling throughput of
        # the otherwise chain-bound recurrence.
        SB = B // 2                                  # 32 rows per stream
        hp = ctx.enter_context(tc.tile_pool(name="h", bufs=1))
        h_tiles = [[hp.tile([65, 3 * SB], MM, tag=f"h{s}{i}", name=f"h{s}{i}")
                    for i in range(2)] for s in range(2)]
        for pair in h_tiles:
            for ht in pair:
                nc.vector.memset(ht[:], 0.0)
                nc.vector.memset(ht[64:65, :], 1.0)   # aug ones row

        psA_pool = ctx.enter_context(
            tc.tile_pool(name="psA", bufs=2, space="PSUM"))
        psB_pool = ctx.enter_context(
            tc.tile_pool(name="psB", bufs=1, space="PSUM"))
        psF_pool = ctx.enter_context(
            tc.tile_pool(name="psF", bufs=1, space="PSUM"))
        sig_pool = ctx.enter_context(tc.tile_pool(name="sig", bufs=3))
        tmp_pool = ctx.enter_context(tc.tile_pool(name="tmp", bufs=3))

        mm = nc.tensor.matmul

        def tick(k, s, wr_lo, wr_hi):
            hc = h_tiles[s][k % 2]
            hn = h_tiles[s][(k + 1) % 2]
            q, c = k % 32, k // 32
            psA = psA_pool.tile([128, 3 * SB], F32, tag=f"psA{s}")
            psB = psB_pool.tile([64, 6 * SB], F32, tag=f"psB{s}")
            xs = xt_sb[0:32, c, SB * s:SB * (s + 1)]          # [32, SB]
            L0, L1, L2 = 0, SB, 2 * SB
            # --- rz args (psA): layer blocks of SB cols ---
            mm(psA[:, L0:L1], lhsT=wx_sb[0:32, q, 0:128], rhs=xs,
               start=True, stop=False)
            mm(psA[:, L0:L1], lhsT=w_ap(wts_sb, "Wh0_rz"),
               rhs=hc[0:65, L0:L1], start=False, stop=True)
            mm(psA[:, L1:L2], lhsT=w_ap(wts_sb, "Wi_rz1"),
               rhs=hc[0:65, L0:L1], start=True, stop=False)
            mm(psA[:, L1:L2], lhsT=w_ap(wts_sb, "Wh_rz1", rows=64),
               rhs=hc[0:64, L1:L2], start=False, stop=True)
            mm(psA[:, L2:3 * SB], lhsT=w_ap(wts_sb, "Wi_rz2"),
               rhs=hc[0:65, L1:L2], start=True, stop=False)
            mm(psA[:, L2:3 * SB], lhsT=w_ap(wts_sb, "Wh_rz2", rows=64),
               rhs=hc[0:64, L2:3 * SB], start=False, stop=True)
            # --- gh_n + b_hn (psB cols 0:3SB) ---
            mm(psB[:, L0:L1], lhsT=w_ap(wts_sb, "Wh0_n"),
               rhs=hc[0:65, L0:L1], start=True, stop=True)
            mm(psB[:, L1:L2], lhsT=w_ap(wts_sb, "Wh_n1"),
               rhs=hc[0:65, L1:L2], start=True, stop=True)
            mm(psB[:, L2:3 * SB], lhsT=w_ap(wts_sb, "Wh_n2"),
               rhs=hc[0:65, L2:3 * SB], start=True, stop=True)
            # --- gi_n + b_in (psB cols 3SB:6SB) ---
            g = 3 * SB
            mm(psB[:, g:g + SB], lhsT=wx_sb[0:32, q, 128:192], rhs=xs,
               start=True, stop=False)
            mm(psB[:, g:g + SB], lhsT=w_ap(wts_sb, "b0_n", rows=1),
               rhs=ones_sb[0:1, 0:SB], start=False, stop=True)
            mm(psB[:, g + SB:g + 2 * SB], lhsT=w_ap(wts_sb, "Wi_n1"),
               rhs=hc[0:65, L0:L1], start=True, stop=True)
            mm(psB[:, g + 2 * SB:g + 3 * SB], lhsT=w_ap(wts_sb, "Wi_n2"),
               rhs=hc[0:65, L1:L2], start=True, stop=True)
            # --- gates ---
            sig = sig_pool.tile([128, 3 * SB], F32, tag=f"sig{s}")
            nc.scalar.activation(sig[:], psA[:], AF.Sigmoid)
            u2 = tmp_pool.tile([64, 3 * SB], F32, tag=f"u2{s}")
            nc.vector.tensor_tensor(u2[:], psB[0:64, 0:g], sig[0:64, :],
                                    op=OP.mult)
            v2 = tmp_pool.tile([64, 3 * SB], F32, tag=f"v2{s}")
            nc.vector.tensor_tensor(v2[:], u2[:], psB[0:64, g:2 * g],
                                    op=OP.add)
            n_t = tmp_pool.tile([64, 3 * SB], F32, tag=f"n{s}")
            nc.scalar.activation(n_t[:], v2[:], AF.Tanh)
            # --- h' = h + w*(n - h) ---
            # C is written into partitions 64:127 so the D multiply reads both
            # inputs (C, w) at base partition 64 — the HW verifier requires
            # equal base partitions for two SBUF inputs; outputs may cross.
            Ct = tmp_pool.tile([128, 3 * SB], F32, tag=f"C{s}")
            nc.gpsimd.tensor_tensor(Ct[64:128, :], n_t[:], hc[0:64, :],
                                    op=OP.subtract)
            Dt = tmp_pool.tile([64, 3 * SB], F32, tag=f"D{s}")
            nc.gpsimd.tensor_tensor(Dt[:], Ct[64:128, :], sig[64:128, :],
                                    op=OP.mult)
            nc.vector.tensor_tensor(hn[0:64, wr_lo:wr_hi],
                                    Dt[:, wr_lo:wr_hi],
                                    hc[0:64, wr_lo:wr_hi], op=OP.add)

        for k in range(n_ticks):
            for s in range(2):
                if k == 0:
                    tick(k, s, 0, SB)
                elif k == 1:
                    tick(k, s, 0, 2 * SB)
                else:
                    tick(k, s, 0, 3 * SB)

        # --- final FC: out[300, B] = fc_w @ h2 + fc_b (per stream) ---
        fco, _ = offs["fc"]
        for s in range(2):
            hfin = h_tiles[s][n_ticks % 2]
            for ci, (mo, mw) in enumerate([(0, 128), (128, 128), (256, 44)]):
                psF = psF_pool.tile([128, SB], F32, tag="psF")
                mm(psF[0:mw, :], lhsT=wts_sb[0:65, fco + mo:fco + mo + mw],
                   rhs=hfin[0:65, 2 * SB:3 * SB], start=True, stop=True)
                ot = tmp_pool.tile([128, SB], F32, tag="fc_out")
                nc.vector.tensor_copy(ot[0:mw, :], psF[0:mw, :])
                nc.sync.dma_start(out=t_out[mo:mo + mw, SB * s:SB * (s + 1)],
                                  in_=ot[0:mw, :])

    nc.compile()
    return nc


def _prep_inputs(inputs, mm_np_dt=np.float32, n_ticks=NT):
    """Host-side shard + repack.  Returns in_maps (one dict per core)."""
    f64in = {k: np.asarray(v, np.float64) for k, v in inputs.items()}
    wts, wx, _ = _build_weights_np(f64in, mm_np_dt)
    x = np.asarray(inputs["x"], np.float64)         # [512, 1000]
    in_maps = []
    for ci in range(NCORES):
        xc = x[ci * B:(ci + 1) * B]                 # [B, T]
        xt = np.zeros((TPAD, B), np.float64)
        xt[:T] = xc.T
        xt = xt.reshape(TPAD // 32, 32, B).transpose(1, 0, 2)
        in_maps.append({
            "xt": np.ascontiguousarray(xt.astype(mm_np_dt)),
            "wx": wx, "wts": wts,
        })
    return in_maps


def _run(inputs, trace=False, mm_np_dt=np.float32, n_ticks=NT):
    from concourse.bass_utils import run_bass_kernel_spmd
    key = (np.dtype(mm_np_dt).name, n_ticks)
    if key not in _cache:
        _cache[key] = _build_program(mm_np_dt, n_ticks)
    nc = _cache[key]
    in_maps = _prep_inputs(inputs, mm_np_dt, n_ticks)
    res = run_bass_kernel_spmd(nc, in_maps, list(range(NCORES)), trace=trace)
    outs = [res.results[i]["out"] for i in range(NCORES)]   # [300, B] each
    full = np.concatenate([o.T for o in outs], axis=0)      # [512, 300]
    return full.astype(np.float32), res


def kernel(**inputs):
    out, _ = _run(inputs, trace=False)
    return out

